# revision 30
# baseline (speedup 1.0000x reference)
"""Trainium2 Bass kernel for nn_Attention (B=2, L=2048, D=1024, H=16).

Returns (output, attn) like the reference:
  qkv = x @ W_qkv ; per-head scores = Q K^T ; attn = softmax(scores/32)
  attn_out = attn @ V ; output = attn_out @ W_h + b_h

Sharding: 8 cores = 2 batches x 4 query-blocks of 512. Each core computes
K/V projections for its full batch (redundantly, avoids all collectives),
scores/softmax/AV for its query block, and the full H projection for its
rows. The host pre-transposes x so no on-chip activation transpose is
needed; the attention matrix is computed in both [q,k] layout (for the
attn output) and [k,q] layout (for the AV matmul contraction).
All matmuls run in float32r (full-rate fp32, ~1.6e-4 component error).
"""

import threading

import numpy as np

import concourse.bass as bass
import concourse.mybir as mybir
import concourse.tile as tile
from concourse import bacc
from concourse.bass_utils import run_bass_kernel_spmd
from concourse.masks import make_identity

B = 2
L = 1024 * 2
D = 1024
H = 16
DH = 64
NCORE = 8
Q = 512            # query rows per core
NPAIR = 8          # head pairs
SCALE = 1.0 / 32.0  # 1/sqrt(D)

F32 = mybir.dt.float32
F32R = mybir.dt.float32r
EXP = mybir.ActivationFunctionType.Exp


def build_bass(nbody: int = 1):
    nc = bacc.Bacc("TRN2", target_bir_lowering=False, debug=False,
                   num_devices=NCORE, dynamic_dma_scratch_size=2048)

    xT = nc.dram_tensor("xT", [D, L], F32R, kind="ExternalInput")
    xqT = nc.dram_tensor("xqT", [D, Q], F32R, kind="ExternalInput")
    wq = nc.dram_tensor("wq", [NPAIR, 128, 8, 128], F32R, kind="ExternalInput")
    wk = nc.dram_tensor("wk", [NPAIR, 128, 8, 128], F32R, kind="ExternalInput")
    wv = nc.dram_tensor("wv", [4, 128, 8, 256], F32R, kind="ExternalInput")
    wh = nc.dram_tensor("wh", [2, 128, 8, 512], F32R, kind="ExternalInput")
    bh = nc.dram_tensor("bh", [128, D], F32, kind="ExternalInput")
    blkd = nc.dram_tensor("blk", [2, 128], F32R, kind="ExternalInput")
    attn = nc.dram_tensor("attn", [H, Q, L], F32, kind="ExternalOutput")
    out = nc.dram_tensor("out", [2, 4, 128, 512], F32, kind="ExternalOutput")

    with tile.TileContext(nc) as tc:
        with (
            tc.tile_pool(name="xt_pool", bufs=1) as xt_pool,
            tc.tile_pool(name="shared", bufs=1) as shared,       # [128,8,512]
            tc.tile_pool(name="qt_pool", bufs=1) as qt_pool,
            tc.tile_pool(name="vg_pool", bufs=1) as vg_pool,
            tc.tile_pool(name="kt_pool", bufs=2) as kt_pool,
            tc.tile_pool(name="w_pool", bufs=2) as w_pool,       # [128,8,128]
            tc.tile_pool(name="exp_pool", bufs=3) as exp_pool,
            tc.tile_pool(name="et_pool", bufs=2) as et_pool,
            tc.tile_pool(name="ao_pool", bufs=8) as ao_pool,
            tc.tile_pool(name="const_pool", bufs=1) as const_pool,
            tc.tile_pool(name="z_pool", bufs=4) as z_pool,
            tc.tile_pool(name="zm_pool", bufs=2) as zm_pool,
            tc.tile_pool(name="rzt_pool", bufs=1) as rzt_pool,
            tc.tile_pool(name="p1", bufs=2, space="PSUM") as p1,
            tc.tile_pool(name="s1", bufs=1, space="PSUM") as s1,
            tc.tile_pool(name="stp", bufs=1, space="PSUM") as stp,
            tc.tile_pool(name="avp", bufs=2, space="PSUM") as avp,
        ):
            ident = const_pool.tile([128, 128], F32, tag="ident")
            make_identity(nc, ident[:])
            bht = const_pool.tile([128, D], F32, tag="bht")
            nc.sync.dma_start(bht[:], bh[:])
            # [2,128] block-indicator: row h = 1 on columns of head h
            blk = const_pool.tile([2, 128], F32R, tag="blk")
            nc.sync.dma_start(blk[:], blkd[:])

            for _ in range(nbody):
                _emit_body(nc, tc, locals())
    nc.compile()
    return nc


def _emit_body(nc, tc, env):
    xT, xqT, wq, wk, wv, wh, attn, out = (env[k] for k in
        ("xT", "xqT", "wq", "wk", "wv", "wh", "attn", "out"))
    xt_pool, shared, qt_pool, vg_pool, kt_pool, w_pool = (env[k] for k in
        ("xt_pool", "shared", "qt_pool", "vg_pool", "kt_pool", "w_pool"))
    exp_pool, et_pool, ao_pool, z_pool, zm_pool, rzt_pool = (env[k] for k in
        ("exp_pool", "et_pool", "ao_pool", "z_pool", "zm_pool", "rzt_pool"))
    p1, s1, stp, avp = (env[k] for k in ("p1", "s1", "stp", "avp"))
    ident, bht, blk = env["ident"], env["bht"], env["blk"]

    # x^T resident: [128, 8 D-slabs, 2048 t]
    xt = xt_pool.tile([128, 8, L], F32R, tag="xt")
    xTr = xT.rearrange("(o p) f -> p o f", p=128)
    for tcc in range(4):
        for d in range(8):
            nc.sync.dma_start(xt[:, d, tcc * 512:(tcc + 1) * 512],
                              xTr[:, d, tcc * 512:(tcc + 1) * 512])

    xq = shared.tile([128, 8, Q], F32R, tag="big")
    nc.sync.dma_start(xq[:], xqT.rearrange("(o p) f -> p o f", p=128))
    qta = qt_pool.tile([128, NPAIR, Q], F32R, tag="qta")

    def emit_qt(p):
        wsq = w_pool.tile([128, 8, 128], F32R, tag="wslab", name=f"wsq{p}")
        nc.sync.dma_start(wsq[:], wq[p])
        ps = p1.tile([128, Q], F32, tag="p1", name=f"qtp{p}")
        for d in range(8):
            nc.tensor.matmul(ps[:], wsq[:, d, :], xq[:, d, :],
                             start=(d == 0), stop=(d == 7))
        nc.vector.tensor_copy(qta[:, p, :], ps[:])

    def emit_v(g2):
        wvs = shared.tile([128, 8, 256], F32R, tag="big", name=f"wvs{g2}")
        nc.sync.dma_start(wvs[:], wv[g2])
        vg = vg_pool.tile([128, 16, 256], F32R, tag="vg", name=f"vg{g2}")
        for t in range(16):
            psv = p1.tile([128, 256], F32, tag="p1", name=f"vps{g2}_{t}")
            for d in range(8):
                nc.tensor.matmul(psv[:], xt[:, d, t * 128:(t + 1) * 128],
                                 wvs[:, d, :], start=(d == 0), stop=(d == 7))
            nc.vector.tensor_copy(vg[:, t, :], psv[:])
        return vg

    # startup: get pair 0's softmax running before anything else
    emit_qt(0)
    kt_cur = _emit_kt(nc, 0, xt, wk, w_pool, kt_pool, p1)
    rz_cur = _emit_pass1(nc, 0, kt_cur, qta, attn, exp_pool, z_pool,
                         zm_pool, s1)
    for p in range(1, NPAIR):
        emit_qt(p)

    aot = [None] * NPAIR
    vg = None
    for p in range(NPAIR):
        lp = p % 4
        if p % 2 == 0:
            vg = emit_v(p // 2)
        if p > 0 and p < NPAIR - 1:
            rz_cur = _emit_pass1(nc, p, kt_cur, qta, attn, exp_pool, z_pool,
                                 zm_pool, s1)
        if p + 1 < NPAIR:
            kt_next = _emit_kt(nc, p + 1, xt, wk, w_pool, kt_pool, p1)
        av = _emit_pass2_core(nc, p, lp, kt_cur, qta, vg, et_pool, stp, avp)
        if p == NPAIR - 1:
            rz_cur = _emit_pass1(nc, p, kt_cur, qta, attn, exp_pool, z_pool,
                                 zm_pool, s1)
        _emit_pass2_fin(nc, p, av, rz_cur, ao_pool, rzt_pool, p1,
                        ident, blk, aot)
        if p + 1 < NPAIR:
            kt_cur = kt_next

    # ---- H projection: out = attn_out @ W_h + b_h ----
    for nc2 in range(2):
        whs = shared.tile([128, 8, 512], F32R, tag="big")
        nc.sync.dma_start(whs[:], wh[nc2])
        for tt in range(4):
            ps = p1.tile([128, 512], F32, tag="p1")
            for p in range(NPAIR):
                nc.tensor.matmul(ps[:], aot[p][:, tt * 128:(tt + 1) * 128],
                                 whs[:, p, :], start=(p == 0), stop=(p == 7))
            ho = exp_pool.tile([128, 512], F32, tag="exp")
            nc.vector.tensor_add(ho[:], ps[:], bht[:, nc2 * 512:(nc2 + 1) * 512])
            nc.sync.dma_start(out[nc2, tt], ho[:])


def _emit_kt(nc, p, xt, wk, w_pool, kt_pool, p1):
    # ---- K^T for pair: [128 c, 2048 k] ----
    ws = w_pool.tile([128, 8, 128], F32R, tag="wslab", name=f"wsk{p}")
    nc.sync.dma_start(ws[:], wk[p])
    kt = kt_pool.tile([128, 2048], F32R, tag="kt")
    for kc in range(4):
        ps = p1.tile([128, 512], F32, tag="p1", name=f"ktp{p}_{kc}")
        for d in range(8):
            nc.tensor.matmul(ps[:], ws[:, d, :],
                             xt[:, d, kc * 512:(kc + 1) * 512],
                             start=(d == 0), stop=(d == 7))
        nc.vector.tensor_copy(kt[:, kc * 512:(kc + 1) * 512], ps[:])
    return kt


def _emit_pass1(nc, p, kt, qta, attn, exp_pool, z_pool, zm_pool, s1):
    # ---- pass 1: scores [q,k], exp + rowsum, normalize, attn out ----
    zmat = zm_pool.tile([128, 4, H], F32, tag="zmat")
    rzmat = zm_pool.tile([128, 4, H], F32, tag="rzmat")
    for qt in range(4):
        qsl = slice(qt * 128, (qt + 1) * 128)
        ex2 = exp_pool.tile([128, 2, 2048], F32, tag="exp", name=f"ex{qt}")
        for h in range(2):
            hsl = slice(h * 64, (h + 1) * 64)
            col = 2 * p + h
            zh = z_pool.tile([128, 2], F32, tag="zh", name=f"zh{qt}_{h}")
            for half in range(2):
                sa = s1.tile([128, 1024], F32, tag="s1", name=f"sa{qt}_{h}_{half}")
                for j in range(2):
                    kc = half * 2 + j
                    ksl = slice(kc * 512, (kc + 1) * 512)
                    jsl = slice(j * 512, (j + 1) * 512)
                    nc.tensor.matmul(sa[:, jsl], qta[hsl, p, qsl],
                                     kt[hsl, ksl], start=True, stop=True,
                                     tile_position=(h * 64, 0))
                fsl = slice(half * 1024, (half + 1) * 1024)
                nc.scalar.activation(ex2[:, h, fsl], sa[:], EXP, scale=SCALE,
                                     accum_out=zh[:, half:half + 1])
            nc.vector.tensor_add(zmat[:, qt, col:col + 1],
                                 zh[:, 0:1], zh[:, 1:2])
            nc.vector.reciprocal(rzmat[:, qt, col:col + 1],
                                 zmat[:, qt, col:col + 1])
            nc.vector.tensor_scalar_mul(ex2[:, h, :], ex2[:, h, :],
                                        rzmat[:, qt, col:col + 1])
        nc.sync.dma_start(
            attn[2 * p:2 * p + 2, qsl, :].rearrange("h q k -> q h k"), ex2[:])
    return rzmat


def _emit_pass2_core(nc, p, lp, kt, qta, vg, et_pool, stp, avp):
    # ---- pass 2: scores^T [k,q], exp FD=1024 covers both heads, AV ----
    av = [avp.tile([64, 512], F32, tag="avp", name=f"av{h}") for h in range(2)]
    for kt_i in range(16):
        ksl = slice(kt_i * 128, (kt_i + 1) * 128)
        st = stp.tile([128, 2, 512], F32, tag="stp", name=f"st{kt_i}")
        et = et_pool.tile([128, 2, 512], F32R, tag="et", name=f"et{kt_i}")
        for h in range(2):
            hsl = slice(h * 64, (h + 1) * 64)
            nc.tensor.matmul(st[:, h, :], kt[hsl, ksl], qta[hsl, p, :],
                             start=True, stop=True,
                             tile_position=(h * 64, 0))
        nc.scalar.activation(et[:], st[:], EXP, scale=SCALE)
        for h in range(2):
            csl = slice((lp % 2) * 128 + h * 64, (lp % 2) * 128 + (h + 1) * 64)
            nc.tensor.matmul(av[h][:], vg[:, kt_i, csl], et[:, h, :],
                             start=(kt_i == 0), stop=(kt_i == 15))
    return av


def _emit_pass2_fin(nc, p, av, rzmat, ao_pool, rzt_pool, p1, ident, blk, aot):
    # ---- normalize attn_out^T by 1/Z (transposed to free axis) ----
    rzt = rzt_pool.tile([2, 512], F32R, tag="rzt")
    for qt in range(4):
        pst = p1.tile([2, 128], F32, tag="p1")
        nc.tensor.transpose(pst[:], rzmat[:, qt, 2 * p:2 * p + 2], ident[:])
        nc.vector.tensor_copy(rzt[:, qt * 128:(qt + 1) * 128], pst[:])
    ao = ao_pool.tile([128, 512], F32R, tag="ao")
    for h in range(2):
        # broadcast 1/Z row h across 64 partitions via a K=2 matmul
        bcp = p1.tile([64, 512], F32, tag="p1", name=f"bcp{h}")
        nc.tensor.matmul(bcp[:], blk[:, h * 64:(h + 1) * 64], rzt[:],
                         start=True, stop=True)
        bcs = rzt_pool.tile([64, 512], F32, tag="bcs", name=f"bcs{h}")
        nc.vector.tensor_copy(bcs[:], bcp[:])
        if h == 0:
            nc.vector.tensor_mul(ao[0:64, :], av[h][:], bcs[:])
        else:
            aob = rzt_pool.tile([64, 512], F32R, tag="aob")
            nc.vector.tensor_mul(aob[:], av[h][:], bcs[:])
            nc.sync.dma_start(ao[64:128, :], aob[:])
    aot[p] = ao


_CACHE = {}
_LOCK = threading.Lock()


def _get_bass(nbody=1):
    with _LOCK:
        if nbody not in _CACHE:
            _CACHE[nbody] = build_bass(nbody)
        return _CACHE[nbody]


def _shard_inputs(x, W_qkv, W_h, b_h):
    def arr_w(w, nblk, csz):
        # [D, D] -> [nblk, 128, 8, csz] : slab for block b, partition p2 holds
        # rows {p2, p2+128, ...} x its block's csz columns
        return np.ascontiguousarray(
            w.reshape(8, 128, nblk, csz).transpose(2, 1, 0, 3))
    wq = arr_w(W_qkv[:, 0:D], NPAIR, 128)
    wk = arr_w(W_qkv[:, D:2 * D], NPAIR, 128)
    wv = arr_w(W_qkv[:, 2 * D:3 * D], 4, 256)
    wh = arr_w(W_h, 2, 512)
    bh = np.ascontiguousarray(np.broadcast_to(b_h[None, :], (128, D)))
    blkm = np.zeros((2, 128), dtype=np.float32)
    blkm[0, 0:64] = 1.0
    blkm[1, 64:128] = 1.0
    xTs = [np.ascontiguousarray(x[b].T) for b in range(B)]
    in_maps = []
    for c in range(NCORE):
        b, qi = divmod(c, 4)
        xT = xTs[b]
        in_maps.append({
            "xT": xT,
            "xqT": np.ascontiguousarray(xT[:, qi * Q:(qi + 1) * Q]),
            "wq": wq, "wk": wk, "wv": wv, "wh": wh, "bh": bh, "blk": blkm,
        })
    return in_maps


def _run(nc, in_maps):
    return run_bass_kernel_spmd(nc, in_maps, core_ids=list(range(NCORE)))


def kernel(x, mask, W_qkv, W_h, b_h, _nbody=1):
    x = np.asarray(x, dtype=np.float32)
    mask = np.asarray(mask, dtype=np.float32)
    W_qkv = np.asarray(W_qkv, dtype=np.float32)
    W_h = np.asarray(W_h, dtype=np.float32)
    b_h = np.asarray(b_h, dtype=np.float32)

    if not np.all(mask == 1.0):
        return _kernel_host_fallback(x, mask, W_qkv, W_h, b_h)

    nc = _get_bass(_nbody)
    in_maps = _shard_inputs(x, W_qkv, W_h, b_h)
    res = _run(nc, in_maps)

    output = np.empty((B, L, D), dtype=np.float32)
    attn = np.empty((B, H, L, L), dtype=np.float32)
    for c in range(NCORE):
        b, qi = divmod(c, 4)
        qsl = slice(qi * Q, (qi + 1) * Q)
        oc = res.results[c]["out"]  # [2, 4, 128, 512]
        output[b, qsl, :] = oc.transpose(1, 2, 0, 3).reshape(Q, D)
        attn[b, :, qsl, :] = res.results[c]["attn"]
    return output, attn


def _kernel_host_fallback(x, mask, W_qkv, W_h, b_h):
    qkv = x @ W_qkv
    Qm, Km, Vm = np.split(qkv, 3, axis=-1)
    Qm = Qm.reshape(B, L, H, DH)
    Km = Km.reshape(B, L, H, DH)
    Vm = Vm.reshape(B, L, H, DH)
    scores = np.einsum("bqhd,bkhd->bhqk", Qm, Km)
    tm = np.einsum("bq,bk->bqk", mask, mask)[:, None, :, :]
    scores = np.where(tm == 0, np.float32(-100000.0), scores)
    s = scores * np.float32(SCALE)
    s = s - s.max(axis=-1, keepdims=True)
    e = np.exp(s)
    attn = e / e.sum(axis=-1, keepdims=True)
    attn_out = np.einsum("bhqv,bvhd->bqhd", attn, Vm)
    output = attn_out.reshape(B, L, D) @ W_h + b_h
    return output.astype(np.float32), attn.astype(np.float32)


# revision 31
# speedup vs baseline: 1.0237x; 1.0237x over previous
"""Trainium2 Bass kernel for nn_Attention (B=2, L=2048, D=1024, H=16).

Returns (output, attn) like the reference:
  qkv = x @ W_qkv ; per-head scores = Q K^T ; attn = softmax(scores/32)
  attn_out = attn @ V ; output = attn_out @ W_h + b_h

Sharding: 8 cores = 2 batches x 4 query-blocks of 512. Each core computes
K/V projections for its full batch (redundantly, avoids all collectives),
scores/softmax/AV for its query block, and the full H projection for its
rows. The host pre-transposes x so no on-chip activation transpose is
needed; the attention matrix is computed in both [q,k] layout (for the
attn output) and [k,q] layout (for the AV matmul contraction).
All matmuls run in float32r (full-rate fp32, ~1.6e-4 component error).
"""

import threading

import numpy as np

import concourse.bass as bass
import concourse.mybir as mybir
import concourse.tile as tile
from concourse import bacc
from concourse.bass_utils import run_bass_kernel_spmd
from concourse.masks import make_identity

B = 2
L = 1024 * 2
D = 1024
H = 16
DH = 64
NCORE = 8
Q = 512            # query rows per core
NPAIR = 8          # head pairs
SCALE = 1.0 / 32.0  # 1/sqrt(D)

F32 = mybir.dt.float32
F32R = mybir.dt.float32r
EXP = mybir.ActivationFunctionType.Exp


def build_bass(nbody: int = 1):
    nc = bacc.Bacc("TRN2", target_bir_lowering=False, debug=False,
                   num_devices=NCORE, dynamic_dma_scratch_size=2048)

    xT = nc.dram_tensor("xT", [D, L], F32R, kind="ExternalInput")
    xqT = nc.dram_tensor("xqT", [D, Q], F32R, kind="ExternalInput")
    wq = nc.dram_tensor("wq", [NPAIR, 128, 8, 128], F32R, kind="ExternalInput")
    wk = nc.dram_tensor("wk", [NPAIR, 128, 8, 128], F32R, kind="ExternalInput")
    wv = nc.dram_tensor("wv", [4, 128, 8, 256], F32R, kind="ExternalInput")
    wh = nc.dram_tensor("wh", [2, 128, 8, 512], F32R, kind="ExternalInput")
    bh = nc.dram_tensor("bh", [128, D], F32, kind="ExternalInput")
    blkd = nc.dram_tensor("blk", [2, 128], F32R, kind="ExternalInput")
    attn = nc.dram_tensor("attn", [H, Q, L], F32, kind="ExternalOutput")
    out = nc.dram_tensor("out", [2, 4, 128, 512], F32, kind="ExternalOutput")

    with tile.TileContext(nc) as tc:
        with (
            tc.tile_pool(name="xt_pool", bufs=1) as xt_pool,
            tc.tile_pool(name="shared", bufs=1) as shared,       # [128,8,512]
            tc.tile_pool(name="qt_pool", bufs=1) as qt_pool,
            tc.tile_pool(name="vg_pool", bufs=1) as vg_pool,
            tc.tile_pool(name="kt_pool", bufs=2) as kt_pool,
            tc.tile_pool(name="w_pool", bufs=2) as w_pool,       # [128,8,128]
            tc.tile_pool(name="exp_pool", bufs=3) as exp_pool,
            tc.tile_pool(name="et_pool", bufs=2) as et_pool,
            tc.tile_pool(name="ao_pool", bufs=8) as ao_pool,
            tc.tile_pool(name="const_pool", bufs=1) as const_pool,
            tc.tile_pool(name="z_pool", bufs=4) as z_pool,
            tc.tile_pool(name="zm_pool", bufs=2) as zm_pool,
            tc.tile_pool(name="rzt_pool", bufs=1) as rzt_pool,
            tc.tile_pool(name="p1", bufs=2, space="PSUM") as p1,
            tc.tile_pool(name="sps", bufs=2, space="PSUM") as sps,
            tc.tile_pool(name="avp", bufs=2, space="PSUM") as avp,
        ):
            ident = const_pool.tile([128, 128], F32, tag="ident")
            make_identity(nc, ident[:])
            bht = const_pool.tile([128, D], F32, tag="bht")
            nc.sync.dma_start(bht[:], bh[:])
            # [2,128] block-indicator: row h = 1 on columns of head h
            blk = const_pool.tile([2, 128], F32R, tag="blk")
            nc.sync.dma_start(blk[:], blkd[:])

            for _ in range(nbody):
                _emit_body(nc, tc, locals())
    nc.compile()
    return nc


def _emit_body(nc, tc, env):
    xT, xqT, wq, wk, wv, wh, attn, out = (env[k] for k in
        ("xT", "xqT", "wq", "wk", "wv", "wh", "attn", "out"))
    xt_pool, shared, qt_pool, vg_pool, kt_pool, w_pool = (env[k] for k in
        ("xt_pool", "shared", "qt_pool", "vg_pool", "kt_pool", "w_pool"))
    exp_pool, et_pool, ao_pool, z_pool, zm_pool, rzt_pool = (env[k] for k in
        ("exp_pool", "et_pool", "ao_pool", "z_pool", "zm_pool", "rzt_pool"))
    p1, sps, avp = (env[k] for k in ("p1", "sps", "avp"))
    s1 = stp = sps
    ident, bht, blk = env["ident"], env["bht"], env["blk"]

    # x^T resident: [128, 8 D-slabs, 2048 t]
    xt = xt_pool.tile([128, 8, L], F32R, tag="xt")
    xTr = xT.rearrange("(o p) f -> p o f", p=128)
    for tcc in range(4):
        for d in range(8):
            nc.sync.dma_start(xt[:, d, tcc * 512:(tcc + 1) * 512],
                              xTr[:, d, tcc * 512:(tcc + 1) * 512])

    xq = shared.tile([128, 8, Q], F32R, tag="big")
    nc.sync.dma_start(xq[:], xqT.rearrange("(o p) f -> p o f", p=128))
    qta = qt_pool.tile([128, NPAIR, Q], F32R, tag="qta")

    def emit_qt(p):
        wsq = w_pool.tile([128, 8, 128], F32R, tag="wslab", name=f"wsq{p}")
        nc.sync.dma_start(wsq[:], wq[p])
        ps = p1.tile([128, Q], F32, tag="p1", name=f"qtp{p}")
        for d in range(8):
            nc.tensor.matmul(ps[:], wsq[:, d, :], xq[:, d, :],
                             start=(d == 0), stop=(d == 7))
        nc.vector.tensor_copy(qta[:, p, :], ps[:])

    def emit_v(g2):
        wvs = shared.tile([128, 8, 256], F32R, tag="big", name=f"wvs{g2}")
        nc.sync.dma_start(wvs[:], wv[g2])
        vg = vg_pool.tile([128, 16, 256], F32R, tag="vg", name=f"vg{g2}")
        for t in range(16):
            psv = p1.tile([128, 256], F32, tag="p1", name=f"vps{g2}_{t}")
            for d in range(8):
                nc.tensor.matmul(psv[:], xt[:, d, t * 128:(t + 1) * 128],
                                 wvs[:, d, :], start=(d == 0), stop=(d == 7))
            nc.vector.tensor_copy(vg[:, t, :], psv[:])
        return vg

    # startup: get pair 0's softmax running before anything else
    emit_qt(0)
    kt_cur = _emit_kt(nc, 0, xt, wk, w_pool, kt_pool, p1)
    rz_cur = _emit_pass1(nc, 0, kt_cur, qta, attn, exp_pool, z_pool,
                         zm_pool, s1)
    for p in range(1, NPAIR):
        emit_qt(p)

    aot = [None] * NPAIR
    vg = None
    for p in range(NPAIR):
        lp = p % 4
        if p % 2 == 0:
            vg = emit_v(p // 2)
        if p > 0 and p < NPAIR - 1:
            rz_cur = _emit_pass1(nc, p, kt_cur, qta, attn, exp_pool, z_pool,
                                 zm_pool, s1)
        if p + 1 < NPAIR:
            kt_next = _emit_kt(nc, p + 1, xt, wk, w_pool, kt_pool, p1)
        av = _emit_pass2_core(nc, p, lp, kt_cur, qta, vg, et_pool, stp, avp)
        if p == NPAIR - 1:
            rz_cur = _emit_pass1(nc, p, kt_cur, qta, attn, exp_pool, z_pool,
                                 zm_pool, s1)
        _emit_pass2_fin(nc, p, av, rz_cur, ao_pool, rzt_pool, p1,
                        ident, blk, aot)
        if p + 1 < NPAIR:
            kt_cur = kt_next

    # ---- H projection: out = attn_out @ W_h + b_h ----
    for nc2 in range(2):
        whs = shared.tile([128, 8, 512], F32R, tag="big")
        nc.sync.dma_start(whs[:], wh[nc2])
        for tt in range(4):
            ps = p1.tile([128, 512], F32, tag="p1")
            for p in range(NPAIR):
                nc.tensor.matmul(ps[:], aot[p][:, tt * 128:(tt + 1) * 128],
                                 whs[:, p, :], start=(p == 0), stop=(p == 7))
            ho = exp_pool.tile([128, 512], F32, tag="exp")
            nc.vector.tensor_add(ho[:], ps[:], bht[:, nc2 * 512:(nc2 + 1) * 512])
            nc.sync.dma_start(out[nc2, tt], ho[:])


def _emit_kt(nc, p, xt, wk, w_pool, kt_pool, p1):
    # ---- K^T for pair: [128 c, 2048 k] ----
    ws = w_pool.tile([128, 8, 128], F32R, tag="wslab", name=f"wsk{p}")
    nc.sync.dma_start(ws[:], wk[p])
    kt = kt_pool.tile([128, 2048], F32R, tag="kt")
    for kc in range(4):
        ps = p1.tile([128, 512], F32, tag="p1", name=f"ktp{p}_{kc}")
        for d in range(8):
            nc.tensor.matmul(ps[:], ws[:, d, :],
                             xt[:, d, kc * 512:(kc + 1) * 512],
                             start=(d == 0), stop=(d == 7))
        nc.vector.tensor_copy(kt[:, kc * 512:(kc + 1) * 512], ps[:])
    return kt


def _emit_pass1(nc, p, kt, qta, attn, exp_pool, z_pool, zm_pool, s1):
    # ---- pass 1: scores [q,k], exp + rowsum, normalize, attn out ----
    zmat = zm_pool.tile([128, 4, H], F32, tag="zmat")
    rzmat = zm_pool.tile([128, 4, H], F32, tag="rzmat")
    for qt in range(4):
        qsl = slice(qt * 128, (qt + 1) * 128)
        ex2 = exp_pool.tile([128, 2, 2048], F32, tag="exp", name=f"ex{qt}")
        for h in range(2):
            hsl = slice(h * 64, (h + 1) * 64)
            col = 2 * p + h
            zh = z_pool.tile([128, 2], F32, tag="zh", name=f"zh{qt}_{h}")
            for half in range(2):
                sa = s1.tile([128, 1024], F32, tag="sps", name=f"sa{qt}_{h}_{half}")
                for j in range(2):
                    kc = half * 2 + j
                    ksl = slice(kc * 512, (kc + 1) * 512)
                    jsl = slice(j * 512, (j + 1) * 512)
                    nc.tensor.matmul(sa[:, jsl], qta[hsl, p, qsl],
                                     kt[hsl, ksl], start=True, stop=True,
                                     tile_position=(h * 64, 0))
                fsl = slice(half * 1024, (half + 1) * 1024)
                nc.scalar.activation(ex2[:, h, fsl], sa[:], EXP, scale=SCALE,
                                     accum_out=zh[:, half:half + 1])
            nc.vector.tensor_add(zmat[:, qt, col:col + 1],
                                 zh[:, 0:1], zh[:, 1:2])
            nc.vector.reciprocal(rzmat[:, qt, col:col + 1],
                                 zmat[:, qt, col:col + 1])
            nc.vector.tensor_scalar_mul(ex2[:, h, :], ex2[:, h, :],
                                        rzmat[:, qt, col:col + 1])
        nc.sync.dma_start(
            attn[2 * p:2 * p + 2, qsl, :].rearrange("h q k -> q h k"), ex2[:])
    return rzmat


def _emit_pass2_core(nc, p, lp, kt, qta, vg, et_pool, stp, avp):
    # ---- pass 2: scores^T [k,q], exp FD=1024 covers both heads, AV ----
    av = [avp.tile([64, 512], F32, tag="avp", name=f"av{h}") for h in range(2)]
    for kt_i in range(16):
        ksl = slice(kt_i * 128, (kt_i + 1) * 128)
        st = stp.tile([128, 2, 512], F32, tag="sps", name=f"st{kt_i}")
        et = et_pool.tile([128, 2, 512], F32R, tag="et", name=f"et{kt_i}")
        for h in range(2):
            hsl = slice(h * 64, (h + 1) * 64)
            nc.tensor.matmul(st[:, h, :], kt[hsl, ksl], qta[hsl, p, :],
                             start=True, stop=True,
                             tile_position=(h * 64, 0))
        nc.scalar.activation(et[:], st[:], EXP, scale=SCALE)
        for h in range(2):
            csl = slice((lp % 2) * 128 + h * 64, (lp % 2) * 128 + (h + 1) * 64)
            nc.tensor.matmul(av[h][:], vg[:, kt_i, csl], et[:, h, :],
                             start=(kt_i == 0), stop=(kt_i == 15))
    return av


def _emit_pass2_fin(nc, p, av, rzmat, ao_pool, rzt_pool, p1, ident, blk, aot):
    # ---- normalize attn_out^T by 1/Z (transposed to free axis) ----
    rzt = rzt_pool.tile([2, 512], F32R, tag="rzt")
    for qt in range(4):
        pst = p1.tile([2, 128], F32, tag="p1")
        nc.tensor.transpose(pst[:], rzmat[:, qt, 2 * p:2 * p + 2], ident[:])
        nc.vector.tensor_copy(rzt[:, qt * 128:(qt + 1) * 128], pst[:])
    ao = ao_pool.tile([128, 512], F32R, tag="ao")
    for h in range(2):
        # broadcast 1/Z row h across 64 partitions via a K=2 matmul
        bcp = p1.tile([64, 512], F32, tag="p1", name=f"bcp{h}")
        nc.tensor.matmul(bcp[:], blk[:, h * 64:(h + 1) * 64], rzt[:],
                         start=True, stop=True)
        bcs = rzt_pool.tile([64, 512], F32, tag="bcs", name=f"bcs{h}")
        nc.vector.tensor_copy(bcs[:], bcp[:])
        if h == 0:
            nc.vector.tensor_mul(ao[0:64, :], av[h][:], bcs[:])
        else:
            aob = rzt_pool.tile([64, 512], F32R, tag="aob")
            nc.vector.tensor_mul(aob[:], av[h][:], bcs[:])
            nc.sync.dma_start(ao[64:128, :], aob[:])
    aot[p] = ao


_CACHE = {}
_LOCK = threading.Lock()


def _get_bass(nbody=1):
    with _LOCK:
        if nbody not in _CACHE:
            _CACHE[nbody] = build_bass(nbody)
        return _CACHE[nbody]


def _shard_inputs(x, W_qkv, W_h, b_h):
    def arr_w(w, nblk, csz):
        # [D, D] -> [nblk, 128, 8, csz] : slab for block b, partition p2 holds
        # rows {p2, p2+128, ...} x its block's csz columns
        return np.ascontiguousarray(
            w.reshape(8, 128, nblk, csz).transpose(2, 1, 0, 3))
    wq = arr_w(W_qkv[:, 0:D], NPAIR, 128)
    wk = arr_w(W_qkv[:, D:2 * D], NPAIR, 128)
    wv = arr_w(W_qkv[:, 2 * D:3 * D], 4, 256)
    wh = arr_w(W_h, 2, 512)
    bh = np.ascontiguousarray(np.broadcast_to(b_h[None, :], (128, D)))
    blkm = np.zeros((2, 128), dtype=np.float32)
    blkm[0, 0:64] = 1.0
    blkm[1, 64:128] = 1.0
    xTs = [np.ascontiguousarray(x[b].T) for b in range(B)]
    in_maps = []
    for c in range(NCORE):
        b, qi = divmod(c, 4)
        xT = xTs[b]
        in_maps.append({
            "xT": xT,
            "xqT": np.ascontiguousarray(xT[:, qi * Q:(qi + 1) * Q]),
            "wq": wq, "wk": wk, "wv": wv, "wh": wh, "bh": bh, "blk": blkm,
        })
    return in_maps


def _run(nc, in_maps):
    return run_bass_kernel_spmd(nc, in_maps, core_ids=list(range(NCORE)))


def kernel(x, mask, W_qkv, W_h, b_h, _nbody=1):
    x = np.asarray(x, dtype=np.float32)
    mask = np.asarray(mask, dtype=np.float32)
    W_qkv = np.asarray(W_qkv, dtype=np.float32)
    W_h = np.asarray(W_h, dtype=np.float32)
    b_h = np.asarray(b_h, dtype=np.float32)

    if not np.all(mask == 1.0):
        return _kernel_host_fallback(x, mask, W_qkv, W_h, b_h)

    nc = _get_bass(_nbody)
    in_maps = _shard_inputs(x, W_qkv, W_h, b_h)
    res = _run(nc, in_maps)

    output = np.empty((B, L, D), dtype=np.float32)
    attn = np.empty((B, H, L, L), dtype=np.float32)
    for c in range(NCORE):
        b, qi = divmod(c, 4)
        qsl = slice(qi * Q, (qi + 1) * Q)
        oc = res.results[c]["out"]  # [2, 4, 128, 512]
        output[b, qsl, :] = oc.transpose(1, 2, 0, 3).reshape(Q, D)
        attn[b, :, qsl, :] = res.results[c]["attn"]
    return output, attn


def _kernel_host_fallback(x, mask, W_qkv, W_h, b_h):
    qkv = x @ W_qkv
    Qm, Km, Vm = np.split(qkv, 3, axis=-1)
    Qm = Qm.reshape(B, L, H, DH)
    Km = Km.reshape(B, L, H, DH)
    Vm = Vm.reshape(B, L, H, DH)
    scores = np.einsum("bqhd,bkhd->bhqk", Qm, Km)
    tm = np.einsum("bq,bk->bqk", mask, mask)[:, None, :, :]
    scores = np.where(tm == 0, np.float32(-100000.0), scores)
    s = scores * np.float32(SCALE)
    s = s - s.max(axis=-1, keepdims=True)
    e = np.exp(s)
    attn = e / e.sum(axis=-1, keepdims=True)
    attn_out = np.einsum("bhqv,bvhd->bqhd", attn, Vm)
    output = attn_out.reshape(B, L, D) @ W_h + b_h
    return output.astype(np.float32), attn.astype(np.float32)


# revision 34
# speedup vs baseline: 1.0567x; 1.0322x over previous
"""Trainium2 Bass kernel for nn_Attention (B=2, L=2048, D=1024, H=16).

Returns (output, attn) like the reference:
  qkv = x @ W_qkv ; per-head scores = Q K^T ; attn = softmax(scores/32)
  attn_out = attn @ V ; output = attn_out @ W_h + b_h

Sharding: 8 cores = 2 batches x 4 query-blocks of 512. Each core computes
K/V projections for its full batch (redundantly, avoids all collectives),
scores/softmax/AV for its query block, and the full H projection for its
rows. The host pre-transposes x so no on-chip activation transpose is
needed; the attention matrix is computed in both [q,k] layout (for the
attn output) and [k,q] layout (for the AV matmul contraction).
All matmuls run in float32r (full-rate fp32, ~1.6e-4 component error).
"""

import threading

import numpy as np

import concourse.bass as bass
import concourse.mybir as mybir
import concourse.tile as tile
from concourse import bacc
from concourse.bass_utils import run_bass_kernel_spmd
from concourse.masks import make_identity

B = 2
L = 1024 * 2
D = 1024
H = 16
DH = 64
NCORE = 8
Q = 512            # query rows per core
NPAIR = 8          # head pairs
SCALE = 1.0 / 32.0  # 1/sqrt(D)

F32 = mybir.dt.float32
F32R = mybir.dt.float32r
EXP = mybir.ActivationFunctionType.Exp


def build_bass(nbody: int = 1):
    nc = bacc.Bacc("TRN2", target_bir_lowering=False, debug=False,
                   num_devices=NCORE, dynamic_dma_scratch_size=2048)

    xT = nc.dram_tensor("xT", [D, L], F32R, kind="ExternalInput")
    xqT = nc.dram_tensor("xqT", [D, Q], F32R, kind="ExternalInput")
    wq = nc.dram_tensor("wq", [NPAIR, 128, 8, 128], F32R, kind="ExternalInput")
    wk = nc.dram_tensor("wk", [NPAIR, 128, 8, 128], F32R, kind="ExternalInput")
    wv = nc.dram_tensor("wv", [4, 128, 8, 256], F32R, kind="ExternalInput")
    wh = nc.dram_tensor("wh", [2, 128, 8, 512], F32R, kind="ExternalInput")
    bh = nc.dram_tensor("bh", [128, D], F32, kind="ExternalInput")
    blkd = nc.dram_tensor("blk", [2, 128], F32R, kind="ExternalInput")
    attn = nc.dram_tensor("attn", [H, Q, L], F32, kind="ExternalOutput")
    out = nc.dram_tensor("out", [2, 4, 128, 512], F32, kind="ExternalOutput")

    with tile.TileContext(nc) as tc:
        with (
            tc.tile_pool(name="xt_pool", bufs=1) as xt_pool,
            tc.tile_pool(name="shared", bufs=1) as shared,       # [128,8,512]
            tc.tile_pool(name="qt_pool", bufs=1) as qt_pool,
            tc.tile_pool(name="vg_pool", bufs=1) as vg_pool,
            tc.tile_pool(name="kt_pool", bufs=2) as kt_pool,
            tc.tile_pool(name="wet", bufs=4) as w_pool,          # [128,8,128]
            tc.tile_pool(name="exp_pool", bufs=3) as exp_pool,
            tc.tile_pool(name="ao_pool", bufs=8) as ao_pool,
            tc.tile_pool(name="const_pool", bufs=1) as const_pool,
            tc.tile_pool(name="z_pool", bufs=4) as z_pool,
            tc.tile_pool(name="zm_pool", bufs=2) as zm_pool,
            tc.tile_pool(name="rzt_pool", bufs=1) as rzt_pool,
            tc.tile_pool(name="p1", bufs=2, space="PSUM") as p1,
            tc.tile_pool(name="sps", bufs=2, space="PSUM") as sps,
            tc.tile_pool(name="avp", bufs=2, space="PSUM") as avp,
        ):
            ident = const_pool.tile([128, 128], F32, tag="ident")
            make_identity(nc, ident[:])
            bht = const_pool.tile([128, D], F32, tag="bht")
            nc.sync.dma_start(bht[:], bh[:])
            # [2,128] block-indicator: row h = 1 on columns of head h
            blk = const_pool.tile([2, 128], F32R, tag="blk")
            nc.sync.dma_start(blk[:], blkd[:])

            for _ in range(nbody):
                _emit_body(nc, tc, locals())
    nc.compile()
    return nc


def _emit_body(nc, tc, env):
    xT, xqT, wq, wk, wv, wh, attn, out = (env[k] for k in
        ("xT", "xqT", "wq", "wk", "wv", "wh", "attn", "out"))
    xt_pool, shared, qt_pool, vg_pool, kt_pool, w_pool = (env[k] for k in
        ("xt_pool", "shared", "qt_pool", "vg_pool", "kt_pool", "w_pool"))
    exp_pool, ao_pool, z_pool, zm_pool, rzt_pool = (env[k] for k in
        ("exp_pool", "ao_pool", "z_pool", "zm_pool", "rzt_pool"))
    et_pool = w_pool
    p1, sps, avp = (env[k] for k in ("p1", "sps", "avp"))
    s1 = stp = sps
    ident, bht, blk = env["ident"], env["bht"], env["blk"]

    # x^T resident: [128, 8 D-slabs, 2048 t]
    xt = xt_pool.tile([128, 8, L], F32R, tag="xt")
    xTr = xT.rearrange("(o p) f -> p o f", p=128)
    for tcc in range(4):
        for d in range(8):
            nc.sync.dma_start(xt[:, d, tcc * 512:(tcc + 1) * 512],
                              xTr[:, d, tcc * 512:(tcc + 1) * 512])

    xq = shared.tile([128, 8, Q], F32R, tag="big")
    nc.sync.dma_start(xq[:], xqT.rearrange("(o p) f -> p o f", p=128))
    qta = qt_pool.tile([128, NPAIR, Q], F32R, tag="qta")

    def emit_qt(p):
        wsq = w_pool.tile([128, 8, 128], F32R, tag="wet", name=f"wsq{p}")
        nc.sync.dma_start(wsq[:], wq[p])
        ps = p1.tile([128, Q], F32, tag="p1", name=f"qtp{p}")
        for d in range(8):
            nc.tensor.matmul(ps[:], wsq[:, d, :], xq[:, d, :],
                             start=(d == 0), stop=(d == 7))
        nc.vector.tensor_copy(qta[:, p, :], ps[:])

    def emit_v(g2):
        wvs = shared.tile([128, 8, 256], F32R, tag="big", name=f"wvs{g2}")
        nc.sync.dma_start(wvs[:], wv[g2])
        vg = vg_pool.tile([128, 16, 256], F32R, tag="vg", name=f"vg{g2}")
        for t in range(16):
            psv = p1.tile([128, 256], F32, tag="p1", name=f"vps{g2}_{t}")
            for d in range(8):
                nc.tensor.matmul(psv[:], xt[:, d, t * 128:(t + 1) * 128],
                                 wvs[:, d, :], start=(d == 0), stop=(d == 7))
            nc.vector.tensor_copy(vg[:, t, :], psv[:])
        return vg

    # startup: get pair 0's softmax running before anything else
    emit_qt(0)
    kt_cur = _emit_kt(nc, 0, xt, wk, w_pool, kt_pool, p1)
    rz_cur = _emit_pass1(nc, 0, kt_cur, qta, attn, exp_pool, z_pool,
                         zm_pool, s1)
    for p in range(1, NPAIR):
        emit_qt(p)

    aot = [None] * NPAIR
    vg = None
    for p in range(NPAIR):
        lp = p % 4
        if p % 2 == 0:
            vg = emit_v(p // 2)
        if p > 0 and p < NPAIR - 1:
            rz_cur = _emit_pass1(nc, p, kt_cur, qta, attn, exp_pool, z_pool,
                                 zm_pool, s1)
        if p + 1 < NPAIR:
            kt_next = _emit_kt(nc, p + 1, xt, wk, w_pool, kt_pool, p1)
        av = _emit_pass2_core(nc, p, lp, kt_cur, qta, vg, et_pool, stp, avp)
        if p == NPAIR - 1:
            rz_cur = _emit_pass1(nc, p, kt_cur, qta, attn, exp_pool, z_pool,
                                 zm_pool, s1)
        _emit_pass2_fin(nc, p, av, rz_cur, ao_pool, rzt_pool, p1,
                        ident, blk, aot)
        if p + 1 < NPAIR:
            kt_cur = kt_next

    # ---- H projection: out = attn_out @ W_h + b_h ----
    for nc2 in range(2):
        whs = shared.tile([128, 8, 512], F32R, tag="big")
        nc.sync.dma_start(whs[:], wh[nc2])
        for tt in range(4):
            ps = p1.tile([128, 512], F32, tag="p1")
            for p in range(NPAIR):
                nc.tensor.matmul(ps[:], aot[p][:, tt * 128:(tt + 1) * 128],
                                 whs[:, p, :], start=(p == 0), stop=(p == 7))
            ho = exp_pool.tile([128, 512], F32, tag="exp")
            nc.vector.tensor_add(ho[:], ps[:], bht[:, nc2 * 512:(nc2 + 1) * 512])
            nc.sync.dma_start(out[nc2, tt], ho[:])


def _emit_kt(nc, p, xt, wk, w_pool, kt_pool, p1):
    # ---- K^T for pair: [128 c, 2048 k] ----
    ws = w_pool.tile([128, 8, 128], F32R, tag="wet", name=f"wsk{p}")
    nc.sync.dma_start(ws[:], wk[p])
    kt = kt_pool.tile([128, 2048], F32R, tag="kt")
    for kc in range(4):
        ps = p1.tile([128, 512], F32, tag="p1", name=f"ktp{p}_{kc}")
        for d in range(8):
            nc.tensor.matmul(ps[:], ws[:, d, :],
                             xt[:, d, kc * 512:(kc + 1) * 512],
                             start=(d == 0), stop=(d == 7))
        nc.vector.tensor_copy(kt[:, kc * 512:(kc + 1) * 512], ps[:])
    return kt


def _emit_pass1(nc, p, kt, qta, attn, exp_pool, z_pool, zm_pool, s1):
    # ---- pass 1: scores [q,k], exp + rowsum, normalize, attn out ----
    zmat = zm_pool.tile([128, 4, H], F32, tag="zmat")
    rzmat = zm_pool.tile([128, 4, H], F32, tag="rzmat")
    for qt in range(4):
        qsl = slice(qt * 128, (qt + 1) * 128)
        ex2 = exp_pool.tile([128, 2, 2048], F32, tag="exp", name=f"ex{qt}")
        for h in range(2):
            hsl = slice(h * 64, (h + 1) * 64)
            col = 2 * p + h
            zh = z_pool.tile([128, 2], F32, tag="zh", name=f"zh{qt}_{h}")
            for half in range(2):
                sa = s1.tile([128, 1024], F32, tag="sps", name=f"sa{qt}_{h}_{half}")
                for j in range(2):
                    kc = half * 2 + j
                    ksl = slice(kc * 512, (kc + 1) * 512)
                    jsl = slice(j * 512, (j + 1) * 512)
                    nc.tensor.matmul(sa[:, jsl], qta[hsl, p, qsl],
                                     kt[hsl, ksl], start=True, stop=True,
                                     tile_position=(h * 64, 0))
                fsl = slice(half * 1024, (half + 1) * 1024)
                nc.scalar.activation(ex2[:, h, fsl], sa[:], EXP, scale=SCALE,
                                     accum_out=zh[:, half:half + 1])
            nc.vector.tensor_add(zmat[:, qt, col:col + 1],
                                 zh[:, 0:1], zh[:, 1:2])
            nc.vector.reciprocal(rzmat[:, qt, col:col + 1],
                                 zmat[:, qt, col:col + 1])
            nc.vector.tensor_scalar_mul(ex2[:, h, :], ex2[:, h, :],
                                        rzmat[:, qt, col:col + 1])
        nc.sync.dma_start(
            attn[2 * p:2 * p + 2, qsl, :].rearrange("h q k -> q h k"), ex2[:])
    return rzmat


def _emit_pass2_core(nc, p, lp, kt, qta, vg, et_pool, stp, avp):
    # ---- pass 2: scores^T [k,q], exp FD=1024 covers both heads, AV ----
    av = [avp.tile([64, 512], F32, tag="avp", name=f"av{h}") for h in range(2)]
    for kt_i in range(16):
        ksl = slice(kt_i * 128, (kt_i + 1) * 128)
        st = stp.tile([128, 2, 512], F32, tag="sps", name=f"st{kt_i}")
        et = et_pool.tile([128, 2, 512], F32R, tag="wet", name=f"et{kt_i}")
        for h in range(2):
            hsl = slice(h * 64, (h + 1) * 64)
            nc.tensor.matmul(st[:, h, :], kt[hsl, ksl], qta[hsl, p, :],
                             start=True, stop=True,
                             tile_position=(h * 64, 0))
        nc.scalar.activation(et[:], st[:], EXP, scale=SCALE)
        for h in range(2):
            csl = slice((lp % 2) * 128 + h * 64, (lp % 2) * 128 + (h + 1) * 64)
            nc.tensor.matmul(av[h][:], vg[:, kt_i, csl], et[:, h, :],
                             start=(kt_i == 0), stop=(kt_i == 15))
    return av


def _emit_pass2_fin(nc, p, av, rzmat, ao_pool, rzt_pool, p1, ident, blk, aot):
    # ---- normalize attn_out^T by 1/Z (transposed to free axis) ----
    rzt = rzt_pool.tile([2, 512], F32R, tag="rzt")
    for qt in range(4):
        pst = p1.tile([2, 128], F32, tag="p1")
        nc.tensor.transpose(pst[:], rzmat[:, qt, 2 * p:2 * p + 2], ident[:])
        nc.vector.tensor_copy(rzt[:, qt * 128:(qt + 1) * 128], pst[:])
    ao = ao_pool.tile([128, 512], F32R, tag="ao")
    for h in range(2):
        # broadcast 1/Z row h across 64 partitions via a K=2 matmul
        bcp = p1.tile([64, 512], F32, tag="p1", name=f"bcp{h}")
        nc.tensor.matmul(bcp[:], blk[:, h * 64:(h + 1) * 64], rzt[:],
                         start=True, stop=True)
        bcs = rzt_pool.tile([64, 512], F32, tag="bcs", name=f"bcs{h}")
        nc.vector.tensor_copy(bcs[:], bcp[:])
        if h == 0:
            nc.vector.tensor_mul(ao[0:64, :], av[h][:], bcs[:])
        else:
            aob = rzt_pool.tile([64, 512], F32R, tag="aob")
            nc.vector.tensor_mul(aob[:], av[h][:], bcs[:])
            nc.sync.dma_start(ao[64:128, :], aob[:])
    aot[p] = ao


_CACHE = {}
_LOCK = threading.Lock()


def _get_bass(nbody=1):
    with _LOCK:
        if nbody not in _CACHE:
            _CACHE[nbody] = build_bass(nbody)
        return _CACHE[nbody]


def _shard_inputs(x, W_qkv, W_h, b_h):
    def arr_w(w, nblk, csz):
        # [D, D] -> [nblk, 128, 8, csz] : slab for block b, partition p2 holds
        # rows {p2, p2+128, ...} x its block's csz columns
        return np.ascontiguousarray(
            w.reshape(8, 128, nblk, csz).transpose(2, 1, 0, 3))
    wq = arr_w(W_qkv[:, 0:D], NPAIR, 128)
    wk = arr_w(W_qkv[:, D:2 * D], NPAIR, 128)
    wv = arr_w(W_qkv[:, 2 * D:3 * D], 4, 256)
    wh = arr_w(W_h, 2, 512)
    bh = np.ascontiguousarray(np.broadcast_to(b_h[None, :], (128, D)))
    blkm = np.zeros((2, 128), dtype=np.float32)
    blkm[0, 0:64] = 1.0
    blkm[1, 64:128] = 1.0
    xTs = [np.ascontiguousarray(x[b].T) for b in range(B)]
    in_maps = []
    for c in range(NCORE):
        b, qi = divmod(c, 4)
        xT = xTs[b]
        in_maps.append({
            "xT": xT,
            "xqT": np.ascontiguousarray(xT[:, qi * Q:(qi + 1) * Q]),
            "wq": wq, "wk": wk, "wv": wv, "wh": wh, "bh": bh, "blk": blkm,
        })
    return in_maps


def _run(nc, in_maps):
    return run_bass_kernel_spmd(nc, in_maps, core_ids=list(range(NCORE)))


def kernel(x, mask, W_qkv, W_h, b_h, _nbody=1):
    x = np.asarray(x, dtype=np.float32)
    mask = np.asarray(mask, dtype=np.float32)
    W_qkv = np.asarray(W_qkv, dtype=np.float32)
    W_h = np.asarray(W_h, dtype=np.float32)
    b_h = np.asarray(b_h, dtype=np.float32)

    if not np.all(mask == 1.0):
        return _kernel_host_fallback(x, mask, W_qkv, W_h, b_h)

    nc = _get_bass(_nbody)
    in_maps = _shard_inputs(x, W_qkv, W_h, b_h)
    res = _run(nc, in_maps)

    output = np.empty((B, L, D), dtype=np.float32)
    attn = np.empty((B, H, L, L), dtype=np.float32)
    for c in range(NCORE):
        b, qi = divmod(c, 4)
        qsl = slice(qi * Q, (qi + 1) * Q)
        oc = res.results[c]["out"]  # [2, 4, 128, 512]
        output[b, qsl, :] = oc.transpose(1, 2, 0, 3).reshape(Q, D)
        attn[b, :, qsl, :] = res.results[c]["attn"]
    return output, attn


def _kernel_host_fallback(x, mask, W_qkv, W_h, b_h):
    qkv = x @ W_qkv
    Qm, Km, Vm = np.split(qkv, 3, axis=-1)
    Qm = Qm.reshape(B, L, H, DH)
    Km = Km.reshape(B, L, H, DH)
    Vm = Vm.reshape(B, L, H, DH)
    scores = np.einsum("bqhd,bkhd->bhqk", Qm, Km)
    tm = np.einsum("bq,bk->bqk", mask, mask)[:, None, :, :]
    scores = np.where(tm == 0, np.float32(-100000.0), scores)
    s = scores * np.float32(SCALE)
    s = s - s.max(axis=-1, keepdims=True)
    e = np.exp(s)
    attn = e / e.sum(axis=-1, keepdims=True)
    attn_out = np.einsum("bhqv,bvhd->bqhd", attn, Vm)
    output = attn_out.reshape(B, L, D) @ W_h + b_h
    return output.astype(np.float32), attn.astype(np.float32)
